# revision 53
# baseline (speedup 1.0000x reference)
"""BatchedLIDIA denoiser on 8 TRN2 NeuronCores — v11.

Sharding: data-parallel over (frame t x row-half), 4*2 = 8 cores; each core
processes 64 query rows x 128 cols x all 225 search offsets.

Same math as v3 (diff^2 -> fp8 DoubleRow box matmuls -> exp / soft-relu
selection -> fold), restructured for instruction-count overhead and engine
balance (engine rates measured on HW: DVE ~0.52ns/col for all-bf16
tensor_tensor (2x mode), ~0.26 for tensor_scalar (4x); ACT 0.83ns/col;
Pool unusable for elementwise beyond ~memsets):

  - Phase A runs per oy row (15 wide ops instead of 225 narrow ones): one
    overlapping-window tensor_sub makes all 15 ox diffs [108,2,15,132]; the
    square into the flat fp8 tap buffer is split 10 slabs on ACT / 5 on DVE
    to balance the two engines; 5x5 box-distance taps stay on PE (fp8
    DoubleRow); exp per 3-offset PSUM group on ACT; per-row soft-relu
    (tensor_scalar, 4x) and a 15-lane bf16 Z-accumulate on DVE.  The
    diff/square production runs 2 rows ahead of the tap/exp consumption
    (diff bufs=3, sq bufs=4, PSUM bufs=4) so PE and ACT never starve.
  - Z = lane tree-fold + reciprocal at the A/B seam (~5us).
  - Phase B per oy row: w = d*(1/Z) scaled in place on DVE right before
    its consumers (on Pool it sat in the critical path; scale+va+tprod+
    pair folds keep DVE ~90% busy and are all at the 2x roofline);
    R = boxT(w) via 3 PE taps per group after the va column pair-sum; one
    15-offset-wide tprod multiply; 15->8->4 pair folds; 4 identity matmuls
    accumulate every row into ONE persistent PSUM bank (start/stop span
    the whole phase), with emission 2 rows deep (ident(oy), boxt(oy+2),
    tprod(oy+1)).
  - pbigA/pbigB row-shift layouts are materialized on the HOST and DMA'd
    as single contiguous 8.7/13.1KB-per-partition transfers (pbigA split
    lo/hi so the first sub starts after the lo half lands).

Host: normalization, reflect-pad, shift-materialization, shard; gather,
overlap-sum, divide by the constant coverage map, un-normalize.
"""
import os
import sys

import numpy as np

sys.path.insert(0, "/opt/trn_rl_repo")

import ml_dtypes  # noqa: E402
from contextlib import ExitStack  # noqa: E402

import concourse.bass as bass  # noqa: E402
import concourse.mybir as mybir  # noqa: E402
import concourse.tile as tile  # noqa: E402
from concourse.bass_utils import run_bass_kernel_spmd  # noqa: E402

PS, WS = 5, 15
SW, PW, RAD = 7, 2, 9
T, C, H, W = 4, 3, 128, 128
HP = H + 2 * PW          # 132
PADHW = H + 2 * RAD      # 146
NOFF = WS * WS           # 225
RH = 64                  # query rows per core
ER = RH + PS - 1         # 68  acc rows per core
PR = ER + WS - 1         # 82  P rows per core
EW = W + 2 * PW          # 132 acc cols
QR = 32                  # query rows per strip
ERS = QR + PS - 1        # 36  sq rows per strip
PCH = C * ERS            # 108 partitions for (ch,row) packing
GA = 3                   # offsets per phase-A PSUM group (5 groups per oy)
SQF = WS * EW + 4        # 1984 flat sq width: 15*132 data + 4 tap-bleed pad
GB = 3                   # offsets per phase-B PSUM group
VPW = 144                # padded per-offset width in the weights buffer
TAU0 = 5e-4              # constant soft-relu threshold (self-match e=1 dominates)
BF16 = mybir.dt.bfloat16
FP8 = mybir.dt.float8e4
F32 = mybir.dt.float32

_CACHE = {}


def _build(neg_inv_denom: float, split_waits: bool = True) -> bass.Bass:
    nc = bass.Bass(target_bir_lowering=False)
    # host-materialized row-shift layouts (oy-major for contiguous slices)
    pa_in = nc.declare_dram_parameter("pa", [PCH, WS, 2, PADHW], BF16,
                                      isOutput=False)
    pb_in = nc.declare_dram_parameter("pb", [ER, WS, C, PADHW], BF16,
                                      isOutput=False)
    bbs_in = nc.declare_dram_parameter("bbs", [PCH, 2, RH], FP8, isOutput=False)
    b2_in = nc.declare_dram_parameter("b2", [RH, ER], BF16, isOutput=False)
    id_in = nc.declare_dram_parameter("ident", [ER, ER], BF16, isOutput=False)
    acc_out = nc.declare_dram_parameter("acc", [ER, C, EW], F32, isOutput=True)

    with tile.TileContext(nc) as tc, ExitStack() as ctx:
        const = ctx.enter_context(tc.tile_pool(name="const", bufs=1))
        work = ctx.enter_context(tc.tile_pool(name="work", bufs=2))
        psum = ctx.enter_context(tc.tile_pool(name="psum", bufs=3, space="PSUM"))

        # pbigA[(ch,rl), oy, s, x] = P[ch, 32*s + rl + oy, x]; host layout
        # matches the SBUF layout exactly, so the DMAs stream 8.7KB/partition
        # contiguous lines at near peak rate.  Split into two tiles (oy 0-7 /
        # 8-14) with separate dependency tracking, so the first sub (reads
        # oy 0 and the oy=7 base row) starts after the lo half lands.
        # host stores oy rows permuted as [7, 0, 1..6, 8..14]; three tiles
        # sized so the first sub (base row 7 + shift row 0) gates on a tiny
        # 2-row transfer
        pbigA_t1 = const.tile([PCH, 2, 2, PADHW], BF16)
        pbigA_t2 = const.tile([PCH, 6, 2, PADHW], BF16)
        pbigA_t3 = const.tile([PCH, 7, 2, PADHW], BF16)
        pat1 = pbigA_t1[:]
        pat2 = pbigA_t2[:]
        pat3 = pbigA_t3[:]
        pa_src = pa_in[:]
        for tl, o0, n in ((pbigA_t1, 0, 2), (pbigA_t2, 2, 6), (pbigA_t3, 8, 7)):
            nc.gpsimd.dma_start(tl[:], bass.AP(
                pa_src.tensor, o0 * 2 * PADHW,
                [[WS * 2 * PADHW, PCH], [1, n * 2 * PADHW]]))

        bbs_sb = const.tile([PCH, 2, RH], FP8)
        nc.gpsimd.dma_start(bbs_sb[:], bbs_in[:])
        b2_sb = const.tile([RH, ER], BF16)
        nc.gpsimd.dma_start(b2_sb[:], b2_in[:])
        id_sb = const.tile([ER, ER], BF16)
        nc.gpsimd.dma_start(id_sb[:], id_in[:])

        # pbigB[y', oy, ch, x] = P[ch, y' + oy, x] for the fold phase.
        pbigB = const.tile([ER, WS, C, PADHW], BF16)
        pbap = pbigB[:]
        nc.gpsimd.dma_start(pbigB[:], pb_in[:])

        vall = const.tile([RH, NOFF, VPW], BF16)
        zacc15 = const.tile([RH, WS, W], BF16)
        zacc = const.tile([RH, W], F32)
        rz = const.tile([RH, W], F32)
        rz16 = const.tile([RH, W], BF16)
        acc_sb = const.tile([ER, C, EW], F32)

        # pad-column zeroing on Pool keeps DVE free for the first sub
        nc.gpsimd.memset(vall[:, :, 0:PW * 2], 0.0)
        nc.gpsimd.memset(vall[:, :, PW * 2 + W:VPW], 0.0)
        nc.gpsimd.memset(zacc15[:], 0.0)

        # base3[(ch,rl), s, ox, x] = P[ch, 32s+rl+SW, SW+x] broadcast over ox
        base3 = bass.AP(pat1.tensor, pat1.offset + SW,
                        [[pat1.ap[0][0], PCH], [PADHW, 2], [0, WS], [1, EW]])

        # ---- Phase A: per-oy batched diff/square; per-group D -> d ----
        def emit_prod(oy):
            diff = work.tile([PCH, 2, WS, EW], BF16, tag="df", bufs=4)
            sq = work.tile([PCH, 2, SQF], FP8, tag="sq", bufs=4)
            sqa = sq[:]
            nc.gpsimd.memset(sq[:, :, WS * EW:SQF], 0.0)
            # shf[(ch,rl), s, ox, x] = P[ch, 32s+rl+oy, ox+x]
            if oy == SW:
                pat, oloc = pat1, 0
            elif oy == 0:
                pat, oloc = pat1, 1
            elif oy < SW:
                pat, oloc = pat2, oy - 1
            else:
                pat, oloc = pat3, oy - 8
            shf = bass.AP(pat.tensor, pat.offset + oloc * 2 * PADHW,
                          [[pat.ap[0][0], PCH], [PADHW, 2], [1, WS], [1, EW]])
            nc.vector.tensor_sub(diff[:], base3, shf)
            # square split by ox slabs: 10 on ACT, 5 on DVE (balances the
            # two engines' per-row budgets; fp8-dst costs DVE its 2x mode)
            na = 10
            sq_a = bass.AP(sqa.tensor, sqa.offset,
                           [[sqa.ap[0][0], PCH], [SQF, 2], [EW, na], [1, EW]])
            sq_d = bass.AP(sqa.tensor, sqa.offset + na * EW,
                           [[sqa.ap[0][0], PCH], [SQF, 2], [EW, WS - na],
                            [1, EW]])
            nc.scalar.square(sq_a, diff[:, :, 0:na, :])
            nc.vector.tensor_mul(sq_d, diff[:, :, na:WS, :],
                                 diff[:, :, na:WS, :])
            return sq

        def emit_groups(oy, sq):
            sqa = sq[:]
            for g in range(PS):
                o0 = oy * WS + g * GA
                ps = psum.tile([RH, GA, EW], F32, tag="pa", bufs=4)
                for q in range(PS):
                    mv = bass.AP(sqa.tensor, sqa.offset + g * GA * EW + q,
                                 [[sqa.ap[0][0], PCH], [SQF, 2], [1, GA * EW]])
                    nc.tensor.matmul(ps[:], bbs_sb[:], mv,
                                     start=(q == 0), stop=(q == PS - 1),
                                     perf_mode=mybir.MatmulPerfMode.DoubleRow)
                vsl = vall[:, o0:o0 + GA, PW * 2:PW * 2 + W]
                nc.scalar.activation(vsl, ps[:, :, 0:W],
                                     mybir.ActivationFunctionType.Exp,
                                     scale=neg_inv_denom)
            # one per-row soft-relu + one per-row Z-lane add, both on DVE
            # (all-bf16 so the 2x mode applies; Pool is ~10x too slow for
            # elementwise work)
            vrow = vall[:, oy * WS:(oy + 1) * WS, PW * 2:PW * 2 + W]
            nc.vector.tensor_scalar(vrow, vrow, TAU0, 0.0,
                                    op0=mybir.AluOpType.subtract,
                                    op1=mybir.AluOpType.max)
            nc.vector.tensor_add(zacc15[:], zacc15[:], vrow)

        # 2-deep software pipeline: the serial sub(DVE) -> square(ACT) chain
        # of row oy+2 overlaps the tap/exp work of rows oy and oy+1
        sq_q = [emit_prod(0), emit_prod(1)]
        for oy in range(WS):
            if oy + 2 < WS:
                sq_q.append(emit_prod(oy + 2))
            emit_groups(oy, sq_q[oy])

        # ---- V = d * (1/Z): Z by lane tree-fold, then per-row scaling into
        # the vsc buffers — row 0 on DVE (unblocks phase B), rest on Pool
        nc.vector.tensor_add(zacc15[:, 0:7], zacc15[:, 0:7], zacc15[:, 8:15])
        nc.vector.tensor_add(zacc15[:, 0:4], zacc15[:, 0:4], zacc15[:, 4:8])
        nc.vector.tensor_add(zacc15[:, 0:2], zacc15[:, 0:2], zacc15[:, 2:4])
        nc.vector.tensor_add(zacc[:], zacc15[:, 0], zacc15[:, 1])
        nc.vector.reciprocal(rz[:], zacc[:])
        nc.vector.tensor_copy(rz16[:], rz[:])
        rzb15 = rz16[:].unsqueeze(1).broadcast_to([RH, WS, W])

        # ---- Phase B: R = boxT(w); tprod = shift(P)*R; offset-sum on PE ----
        pacc = psum.tile([ER, C, EW], F32, tag="pacc", bufs=1)

        def emit_boxt(oy):
            # w = d * (1/Z) for this row, in place on DVE (cheap there: the
            # whole-row mul is ~1us; on Pool it was 6.6us inside the
            # per-row critical path)
            vsc = vall[:, oy * WS:(oy + 1) * WS, :]
            vslr = vall[:, oy * WS:(oy + 1) * WS, PW * 2:PW * 2 + W]
            nc.vector.tensor_mul(vslr, vslr, rzb15)
            # va[u] = V[u] + V[u-2] (pair-sum) so the 5-tap column box
            # becomes 3 taps: R(x) = va(4+x) + va(3+x) + V(x-4)
            va = work.tile([RH, WS, VPW], BF16, tag="va", bufs=3)
            nc.vector.tensor_add(va[:, :, 2:VPW], vsc[:, :, 2:VPW],
                                 vsc[:, :, 0:VPW - 2])
            rsb = work.tile([ER, WS, EW], BF16, tag="rsb", bufs=3)
            for gb in range(WS // GB):
                ps = psum.tile([ER, GB, EW], F32, tag="pb", bufs=3)
                taps = (va[:, gb * GB:gb * GB + GB, 4:4 + EW],
                        va[:, gb * GB:gb * GB + GB, 3:3 + EW],
                        vsc[:, gb * GB:gb * GB + GB, 0:EW])
                for q, mov in enumerate(taps):
                    nc.tensor.matmul(ps[:], b2_sb[:], mov,
                                     start=(q == 0), stop=(q == 2))
                nc.scalar.mul(rsb[:, gb * GB:gb * GB + GB, :], ps[:], 1.0)
            return rsb

        def emit_tprod(oy, rsb):
            # one 15-offset-wide multiply via overlapping-window AP:
            # tprodw[y', ch, ox, x] = P[ch, y'+oy, ox+x] * R[y', ox, x]
            tprodw = work.tile([ER, C, WS, EW], BF16, tag="tp", bufs=3)
            psh = bass.AP(pbap.tensor, pbap.offset + oy * C * PADHW,
                          [[pbap.ap[0][0], ER], [PADHW, C], [1, WS], [1, EW]])
            rb = rsb[:].unsqueeze(1).broadcast_to([ER, C, WS, EW])
            nc.vector.tensor_mul(tprodw[:], psh, rb)
            return tprodw

        def emit_ident(oy, tprodw):
            # pre-pair tprods on DVE (15 -> 8 -> 4 slots), then 4
            # identity-stationary matmuls accumulate into the persistent bank
            # fold 15 -> 6 slots (fewer DVE columns than 15 -> 4; the two
            # extra identity matmuls ride PE's slack)
            nc.vector.tensor_add(tprodw[:, :, 0:6, :], tprodw[:, :, 0:6, :],
                                 tprodw[:, :, 6:12, :])
            nc.vector.tensor_add(tprodw[:, :, 0:3, :], tprodw[:, :, 0:3, :],
                                 tprodw[:, :, 12:15, :])
            for ox in range(6):
                nc.tensor.matmul(pacc[:], id_sb[:], tprodw[:, :, ox, :],
                                 start=(oy == 0 and ox == 0),
                                 stop=(oy == WS - 1 and ox == 5),
                                 skip_group_check=True)

        rsbs = [emit_boxt(0), emit_boxt(1), emit_boxt(2)]
        tps = [emit_tprod(0, rsbs[0])]
        for oy in range(WS):
            emit_ident(oy, tps[oy])
            if oy + 3 < WS:
                rsbs.append(emit_boxt(oy + 3))
            if oy + 1 < WS:
                tps.append(emit_tprod(oy + 1, rsbs[oy + 1]))

        nc.scalar.mul(acc_sb[:], pacc[:], 1.0)
        nc.gpsimd.dma_start(acc_out[:], acc_sb[:])
    if split_waits:
        _split_multi_waits(nc)
    return nc


def _split_multi_waits(nc: bass.Bass) -> None:
    """walrus codegen accepts one embedded sync-wait per TPB instruction;
    hoist extra waits onto same-engine NoOps placed right before."""
    n = 0
    for f in nc.m.functions:
        for b in f.blocks:
            out = []
            for inst in b.instructions:
                si = getattr(inst, "sync_info", None)
                eng = getattr(inst, "engine", None)
                if (si is not None and si.on_wait and len(si.on_wait) > 1
                        and eng is not None):
                    for w in si.on_wait[:-1]:
                        n += 1
                        out.append(mybir.InstNoOp(
                            name=f"wsplit-{n}-{inst.name}",
                            engine=eng,
                            bass_nofuse=True,
                            sync_info=mybir.SyncInfo(on_wait=[w], on_update=[]),
                        ))
                    si.on_wait = [si.on_wait[-1]]
                out.append(inst)
            b.instructions = out


def _coverage() -> np.ndarray:
    reach = np.zeros(HP, np.float32)
    # count of i in [0,H) with z-4 <= i <= z
    for z in range(HP):
        lo, hi = max(z - (PS - 1), 0), min(z, H - 1)
        reach[z] = max(hi - lo + 1, 0)
    return np.outer(reach, reach)


def kernel(noisy: np.ndarray, sigma: np.ndarray) -> np.ndarray:
    noisy = np.asarray(noisy, np.float32)
    sigma = np.asarray(sigma, np.float32)
    x = (noisy / 255.0 - 0.5) / 0.5
    means = x.mean((-2, -1), keepdims=True)
    x = x - means
    P = np.pad(x, ((0, 0), (0, 0), (RAD, RAD), (RAD, RAD)), mode="reflect")
    Pb = P.astype(ml_dtypes.bfloat16)

    sig = float(sigma[0]) / 255.0 / 0.5
    denom = 2.0 * (C * PS * PS) * (sig * sig) + 1e-8
    key = round(-1.0 / denom, 9)
    if key not in _CACHE:
        _CACHE[key] = _build(key)
    nc = _CACHE[key]

    # bbs[(ch, rl), s, 32*s + rp] = 1 iff 0 <= rl - rp < 5
    rl = np.arange(ERS)
    rp = np.arange(QR)
    band = ((rl[:, None] - rp[None, :] >= 0)
            & (rl[:, None] - rp[None, :] < PS))  # [36, 32]
    bbs = np.zeros((C, ERS, 2, RH), ml_dtypes.float8_e4m3)
    for s in range(2):
        bbs[:, :, s, QR * s:QR * (s + 1)] = band[None]
    bbs = np.ascontiguousarray(bbs.reshape(PCH, 2, RH))
    # b2[r, y'] = 1 iff 0 <= y' - r < 5
    r = np.arange(RH)
    yy = np.arange(ER)
    b2 = ((yy[None, :] - r[:, None] >= 0)
          & (yy[None, :] - r[:, None] < PS)).astype(ml_dtypes.bfloat16)
    b2 = np.ascontiguousarray(b2)
    ident = np.eye(ER, dtype=ml_dtypes.bfloat16)

    # host-side shift materialization (oy-major layouts)
    ch_i = np.arange(C)[:, None, None, None]
    rl_i = np.arange(ERS)[None, :, None, None]
    oy_i = np.arange(WS)[None, None, :, None]
    s_i = np.arange(2)[None, None, None, :]
    rowsA = QR * s_i + rl_i + oy_i            # [1,36,15,2]
    yp_i = np.arange(ER)[:, None, None]
    oyB_i = np.arange(WS)[None, :, None]
    cB_i = np.arange(C)[None, None, :]

    in_maps = []
    for core in range(8):
        t, half = divmod(core, 2)
        r0 = half * RH
        p_loc = Pb[t, :, r0:r0 + PR, :]       # [C, 82, 146]
        pa_host = p_loc[ch_i, rowsA, :].reshape(PCH, WS, 2, PADHW)
        perm = [SW] + [o for o in range(WS) if o != SW]
        pa_host = np.ascontiguousarray(pa_host[:, perm])
        pb_host = np.ascontiguousarray(
            p_loc[cB_i, yp_i + oyB_i, :].transpose(0, 1, 2, 3))  # [68,15,3,146]
        in_maps.append({"pa": pa_host, "pb": pb_host, "bbs": bbs, "b2": b2,
                        "ident": ident})

    trace = bool(int(os.environ.get("KERNEL_TRACE", "0")))
    if trace:
        try:
            import antenv.axon_hooks  # noqa: F401
        except ImportError:
            # This image's antenv lacks axon_hooks; provide the hook via the
            # boot machinery so bass_utils can capture NTFF profiles.
            import types
            from trn_agent_boot.trn_boot import _ntff_profile_via_ctypes
            mod = types.ModuleType("antenv.axon_hooks")
            hook = _ntff_profile_via_ctypes("/opt/axon/libaxon_pjrt.so")
            mod.get_axon_ntff_profile_hook = lambda: hook
            sys.modules["antenv.axon_hooks"] = mod
    res = run_bass_kernel_spmd(nc, in_maps, core_ids=list(range(8)), trace=trace)
    if trace:
        print(f"HW exec time: {res.exec_time_ns} ns")
        kernel.last_exec_time_ns = res.exec_time_ns
        kernel.last_profile = res.profile_json

    full = np.zeros((T, HP, C, HP), np.float32)
    for core in range(8):
        t, half = divmod(core, 2)
        r0 = half * RH
        full[t, r0:r0 + ER] += res.results[core]["acc"]
    full = full.transpose(0, 2, 1, 3)  # [T, C, HP, HP]

    cnt = _coverage()
    deno = full / (cnt[None, None] + 1e-10)
    deno = deno[:, :, PW:PW + H, PW:PW + W]
    deno = deno + means
    return np.asarray(255.0 * (deno * 0.5 + 0.5), np.float32)


if __name__ == "__main__":
    noisy = np.load("/root/problem/noisy.npy")
    sigma = np.load("/root/problem/sigma.npy")
    out = kernel(noisy=noisy, sigma=sigma)
    expected = np.load("/root/problem/expected.npy")
    rel = np.linalg.norm(out - expected) / np.linalg.norm(expected)
    print(f"Relative error vs expected: {rel:.3e}")


# revision 54
# speedup vs baseline: 1.2018x; 1.2018x over previous
"""BatchedLIDIA denoiser on 8 TRN2 NeuronCores — v11.

Sharding: data-parallel over (frame t x row-half), 4*2 = 8 cores; each core
processes 64 query rows x 128 cols x all 225 search offsets.

Same math as v3 (diff^2 -> fp8 DoubleRow box matmuls -> exp / soft-relu
selection -> fold), restructured for instruction-count overhead and engine
balance (engine rates measured on HW: DVE ~0.52ns/col for all-bf16
tensor_tensor (2x mode), ~0.26 for tensor_scalar (4x); ACT 0.83ns/col;
Pool unusable for elementwise beyond ~memsets):

  - Phase A runs per oy row (15 wide ops instead of 225 narrow ones): one
    overlapping-window tensor_sub makes all 15 ox diffs [108,2,15,132]; the
    square into the flat fp8 tap buffer is split 10 slabs on ACT / 5 on DVE
    to balance the two engines; 5x5 box-distance taps stay on PE (fp8
    DoubleRow); exp per 3-offset PSUM group on ACT; per-row soft-relu
    (tensor_scalar, 4x) and a 15-lane bf16 Z-accumulate on DVE.  The
    diff/square production runs 2 rows ahead of the tap/exp consumption
    (diff bufs=3, sq bufs=4, PSUM bufs=4) so PE and ACT never starve.
  - Z = lane tree-fold + reciprocal at the A/B seam (~5us).
  - Phase B per oy row: w = d*(1/Z) scaled in place on DVE right before
    its consumers (on Pool it sat in the critical path; scale+va+tprod+
    pair folds keep DVE ~90% busy and are all at the 2x roofline);
    R = boxT(w) via 3 PE taps per group after the va column pair-sum; one
    15-offset-wide tprod multiply; 15->8->4 pair folds; 4 identity matmuls
    accumulate every row into ONE persistent PSUM bank (start/stop span
    the whole phase), with emission 2 rows deep (ident(oy), boxt(oy+2),
    tprod(oy+1)).
  - pbigA/pbigB row-shift layouts are materialized on the HOST and DMA'd
    as single contiguous 8.7/13.1KB-per-partition transfers (pbigA split
    lo/hi so the first sub starts after the lo half lands).

Host: normalization, reflect-pad, shift-materialization, shard; gather,
overlap-sum, divide by the constant coverage map, un-normalize.
"""
import os
import sys

import numpy as np

sys.path.insert(0, "/opt/trn_rl_repo")

import ml_dtypes  # noqa: E402
from contextlib import ExitStack  # noqa: E402

import concourse.bass as bass  # noqa: E402
import concourse.mybir as mybir  # noqa: E402
import concourse.tile as tile  # noqa: E402
from concourse.bass_utils import run_bass_kernel_spmd  # noqa: E402

PS, WS = 5, 15
SW, PW, RAD = 7, 2, 9
T, C, H, W = 4, 3, 128, 128
HP = H + 2 * PW          # 132
PADHW = H + 2 * RAD      # 146
NOFF = WS * WS           # 225
RH = 64                  # query rows per core
ER = RH + PS - 1         # 68  acc rows per core
PR = ER + WS - 1         # 82  P rows per core
EW = W + 2 * PW          # 132 acc cols
QR = 32                  # query rows per strip
ERS = QR + PS - 1        # 36  sq rows per strip
PCH = C * ERS            # 108 partitions for (ch,row) packing
GA = 3                   # offsets per phase-A PSUM group (5 groups per oy)
SQF = WS * EW + 4        # 1984 flat sq width: 15*132 data + 4 tap-bleed pad
GB = 3                   # offsets per phase-B PSUM group
VPW = 144                # padded per-offset width in the weights buffer
TAU0 = 5e-4              # constant soft-relu threshold (self-match e=1 dominates)
BF16 = mybir.dt.bfloat16
FP8 = mybir.dt.float8e4
F32 = mybir.dt.float32

_CACHE = {}


def _build(neg_inv_denom: float, split_waits: bool = True) -> bass.Bass:
    nc = bass.Bass(target_bir_lowering=False)
    # host-materialized row-shift layouts (oy-major for contiguous slices)
    pa_in = nc.declare_dram_parameter("pa", [PCH, WS, 2, PADHW], BF16,
                                      isOutput=False)
    pb_in = nc.declare_dram_parameter("pb", [ER, WS, C, PADHW], BF16,
                                      isOutput=False)
    bbs_in = nc.declare_dram_parameter("bbs", [PCH, 2, RH], FP8, isOutput=False)
    b2_in = nc.declare_dram_parameter("b2", [RH, ER], BF16, isOutput=False)
    id_in = nc.declare_dram_parameter("ident", [ER, ER], BF16, isOutput=False)
    acc_out = nc.declare_dram_parameter("acc", [ER, C, EW], F32, isOutput=True)

    with tile.TileContext(nc) as tc, ExitStack() as ctx:
        const = ctx.enter_context(tc.tile_pool(name="const", bufs=1))
        work = ctx.enter_context(tc.tile_pool(name="work", bufs=2))
        psum = ctx.enter_context(tc.tile_pool(name="psum", bufs=3, space="PSUM"))

        # pbigA[(ch,rl), oy, s, x] = P[ch, 32*s + rl + oy, x]; host layout
        # matches the SBUF layout exactly, so the DMAs stream 8.7KB/partition
        # contiguous lines at near peak rate.  Split into two tiles (oy 0-7 /
        # 8-14) with separate dependency tracking, so the first sub (reads
        # oy 0 and the oy=7 base row) starts after the lo half lands.
        # host stores oy rows permuted as [7, 0, 1..6, 8..14]; three tiles
        # sized so the first sub (base row 7 + shift row 0) gates on a tiny
        # 2-row transfer
        pbigA_t1 = const.tile([PCH, 2, 2, PADHW], BF16)
        pbigA_t2 = const.tile([PCH, 6, 2, PADHW], BF16)
        pbigA_t3 = const.tile([PCH, 7, 2, PADHW], BF16)
        pat1 = pbigA_t1[:]
        pat2 = pbigA_t2[:]
        pat3 = pbigA_t3[:]
        pa_src = pa_in[:]
        for tl, o0, n in ((pbigA_t1, 0, 2), (pbigA_t2, 2, 6), (pbigA_t3, 8, 7)):
            nc.gpsimd.dma_start(tl[:], bass.AP(
                pa_src.tensor, o0 * 2 * PADHW,
                [[WS * 2 * PADHW, PCH], [1, n * 2 * PADHW]]))

        bbs_sb = const.tile([PCH, 2, RH], FP8)
        nc.gpsimd.dma_start(bbs_sb[:], bbs_in[:])
        b2_sb = const.tile([RH, ER], BF16)
        nc.gpsimd.dma_start(b2_sb[:], b2_in[:])
        id_sb = const.tile([ER, ER], BF16)
        nc.gpsimd.dma_start(id_sb[:], id_in[:])

        # pbigB[y', oy, ch, x] = P[ch, y' + oy, x] for the fold phase.
        pbigB = const.tile([ER, WS, C, PADHW], BF16)
        pbap = pbigB[:]
        nc.gpsimd.dma_start(pbigB[:], pb_in[:])

        vall = const.tile([RH, NOFF, VPW], BF16)
        zacc15 = const.tile([RH, WS, W], BF16)
        zacc = const.tile([RH, W], F32)
        rz = const.tile([RH, W], F32)
        rz16 = const.tile([RH, W], BF16)
        acc_sb = const.tile([ER, C, EW], F32)

        # pad-column zeroing on Pool keeps DVE free for the first sub
        nc.gpsimd.memset(vall[:, :, 0:PW * 2], 0.0)
        nc.gpsimd.memset(vall[:, :, PW * 2 + W:VPW], 0.0)
        nc.gpsimd.memset(zacc15[:], 0.0)

        # base3[(ch,rl), s, ox, x] = P[ch, 32s+rl+SW, SW+x] broadcast over ox
        base3 = bass.AP(pat1.tensor, pat1.offset + SW,
                        [[pat1.ap[0][0], PCH], [PADHW, 2], [0, WS], [1, EW]])

        # ---- Phase A: per-oy batched diff/square; per-group D -> d ----
        def emit_prod(oy):
            diff = work.tile([PCH, 2, WS, EW], BF16, tag="df", bufs=4)
            sq = work.tile([PCH, 2, SQF], FP8, tag="sq", bufs=4)
            sqa = sq[:]
            nc.gpsimd.memset(sq[:, :, WS * EW:SQF], 0.0)
            # shf[(ch,rl), s, ox, x] = P[ch, 32s+rl+oy, ox+x]
            if oy == SW:
                pat, oloc = pat1, 0
            elif oy == 0:
                pat, oloc = pat1, 1
            elif oy < SW:
                pat, oloc = pat2, oy - 1
            else:
                pat, oloc = pat3, oy - 8
            shf = bass.AP(pat.tensor, pat.offset + oloc * 2 * PADHW,
                          [[pat.ap[0][0], PCH], [PADHW, 2], [1, WS], [1, EW]])
            nc.vector.tensor_sub(diff[:], base3, shf)
            # square split by ox slabs: 10 on ACT, 5 on DVE (balances the
            # two engines' per-row budgets; fp8-dst costs DVE its 2x mode)
            na = 10
            sq_a = bass.AP(sqa.tensor, sqa.offset,
                           [[sqa.ap[0][0], PCH], [SQF, 2], [EW, na], [1, EW]])
            sq_d = bass.AP(sqa.tensor, sqa.offset + na * EW,
                           [[sqa.ap[0][0], PCH], [SQF, 2], [EW, WS - na],
                            [1, EW]])
            nc.scalar.square(sq_a, diff[:, :, 0:na, :])
            nc.vector.tensor_mul(sq_d, diff[:, :, na:WS, :],
                                 diff[:, :, na:WS, :])
            return sq

        def emit_groups(oy, sq):
            sqa = sq[:]
            for g in range(PS):
                o0 = oy * WS + g * GA
                ps = psum.tile([RH, GA, EW], F32, tag="pa", bufs=4)
                for q in range(PS):
                    mv = bass.AP(sqa.tensor, sqa.offset + g * GA * EW + q,
                                 [[sqa.ap[0][0], PCH], [SQF, 2], [1, GA * EW]])
                    nc.tensor.matmul(ps[:], bbs_sb[:], mv,
                                     start=(q == 0), stop=(q == PS - 1),
                                     perf_mode=mybir.MatmulPerfMode.DoubleRow)
                vsl = vall[:, o0:o0 + GA, PW * 2:PW * 2 + W]
                nc.scalar.activation(vsl, ps[:, :, 0:W],
                                     mybir.ActivationFunctionType.Exp,
                                     scale=neg_inv_denom)
            # one per-row soft-relu + one per-row Z-lane add, both on DVE
            # (all-bf16 so the 2x mode applies; Pool is ~10x too slow for
            # elementwise work)
            vrow = vall[:, oy * WS:(oy + 1) * WS, PW * 2:PW * 2 + W]
            nc.vector.tensor_scalar(vrow, vrow, TAU0, 0.0,
                                    op0=mybir.AluOpType.subtract,
                                    op1=mybir.AluOpType.max)
            nc.vector.tensor_add(zacc15[:], zacc15[:], vrow)

        # 2-deep software pipeline: the serial sub(DVE) -> square(ACT) chain
        # of row oy+2 overlaps the tap/exp work of rows oy and oy+1
        sq_q = [emit_prod(0), emit_prod(1)]
        for oy in range(WS):
            if oy + 2 < WS:
                sq_q.append(emit_prod(oy + 2))
            emit_groups(oy, sq_q[oy])

        # ---- V = d * (1/Z): Z by lane tree-fold, then per-row scaling into
        # the vsc buffers — row 0 on DVE (unblocks phase B), rest on Pool
        nc.vector.tensor_add(zacc15[:, 0:7], zacc15[:, 0:7], zacc15[:, 8:15])
        nc.vector.tensor_add(zacc15[:, 0:4], zacc15[:, 0:4], zacc15[:, 4:8])
        nc.vector.tensor_add(zacc15[:, 0:2], zacc15[:, 0:2], zacc15[:, 2:4])
        nc.vector.tensor_add(zacc[:], zacc15[:, 0], zacc15[:, 1])
        nc.vector.reciprocal(rz[:], zacc[:])
        nc.vector.tensor_copy(rz16[:], rz[:])
        rzb15 = rz16[:].unsqueeze(1).broadcast_to([RH, WS, W])

        # ---- Phase B: R = boxT(w); tprod = shift(P)*R; offset-sum on PE ----
        pacc = psum.tile([ER, C, EW], F32, tag="pacc", bufs=1)

        def emit_boxt(oy):
            # w = d * (1/Z) for this row, in place on DVE (cheap there: the
            # whole-row mul is ~1us; on Pool it was 6.6us inside the
            # per-row critical path)
            vsc = vall[:, oy * WS:(oy + 1) * WS, :]
            vslr = vall[:, oy * WS:(oy + 1) * WS, PW * 2:PW * 2 + W]
            nc.vector.tensor_mul(vslr, vslr, rzb15)
            # va[u] = V[u] + V[u-2] (pair-sum) so the 5-tap column box
            # becomes 3 taps: R(x) = va(4+x) + va(3+x) + V(x-4)
            va = work.tile([RH, WS, VPW], BF16, tag="va", bufs=3)
            nc.vector.tensor_add(va[:, :, 2:VPW], vsc[:, :, 2:VPW],
                                 vsc[:, :, 0:VPW - 2])
            rsb = work.tile([ER, WS, EW], BF16, tag="rsb", bufs=3)
            for gb in range(WS // GB):
                ps = psum.tile([ER, GB, EW], F32, tag="pb", bufs=2)
                taps = (va[:, gb * GB:gb * GB + GB, 4:4 + EW],
                        va[:, gb * GB:gb * GB + GB, 3:3 + EW],
                        vsc[:, gb * GB:gb * GB + GB, 0:EW])
                for q, mov in enumerate(taps):
                    nc.tensor.matmul(ps[:], b2_sb[:], mov,
                                     start=(q == 0), stop=(q == 2))
                nc.scalar.mul(rsb[:, gb * GB:gb * GB + GB, :], ps[:], 1.0)
            return rsb

        def emit_tprod(oy, rsb):
            # one 15-offset-wide multiply via overlapping-window AP:
            # tprodw[y', ch, ox, x] = P[ch, y'+oy, ox+x] * R[y', ox, x]
            tprodw = work.tile([ER, C, WS, EW], BF16, tag="tp", bufs=3)
            psh = bass.AP(pbap.tensor, pbap.offset + oy * C * PADHW,
                          [[pbap.ap[0][0], ER], [PADHW, C], [1, WS], [1, EW]])
            rb = rsb[:].unsqueeze(1).broadcast_to([ER, C, WS, EW])
            nc.vector.tensor_mul(tprodw[:], psh, rb)
            return tprodw

        def emit_ident(oy, tprodw):
            # pre-pair tprods on DVE (15 -> 8 -> 4 slots), then 4
            # identity-stationary matmuls accumulate into the persistent bank
            # fold 15 -> 6 slots (fewer DVE columns than 15 -> 4; the two
            # extra identity matmuls ride PE's slack)
            nc.vector.tensor_add(tprodw[:, :, 0:6, :], tprodw[:, :, 0:6, :],
                                 tprodw[:, :, 6:12, :])
            nc.vector.tensor_add(tprodw[:, :, 0:3, :], tprodw[:, :, 0:3, :],
                                 tprodw[:, :, 12:15, :])
            for ox in range(6):
                nc.tensor.matmul(pacc[:], id_sb[:], tprodw[:, :, ox, :],
                                 start=(oy == 0 and ox == 0),
                                 stop=(oy == WS - 1 and ox == 5),
                                 skip_group_check=True)

        rsbs = [emit_boxt(0), emit_boxt(1), emit_boxt(2)]
        tps = [emit_tprod(0, rsbs[0])]
        for oy in range(WS):
            emit_ident(oy, tps[oy])
            if oy + 3 < WS:
                rsbs.append(emit_boxt(oy + 3))
            if oy + 1 < WS:
                tps.append(emit_tprod(oy + 1, rsbs[oy + 1]))

        nc.scalar.mul(acc_sb[:], pacc[:], 1.0)
        nc.gpsimd.dma_start(acc_out[:], acc_sb[:])
    if split_waits:
        _split_multi_waits(nc)
    return nc


def _split_multi_waits(nc: bass.Bass) -> None:
    """walrus codegen accepts one embedded sync-wait per TPB instruction;
    hoist extra waits onto same-engine NoOps placed right before."""
    n = 0
    for f in nc.m.functions:
        for b in f.blocks:
            out = []
            for inst in b.instructions:
                si = getattr(inst, "sync_info", None)
                eng = getattr(inst, "engine", None)
                if (si is not None and si.on_wait and len(si.on_wait) > 1
                        and eng is not None):
                    for w in si.on_wait[:-1]:
                        n += 1
                        out.append(mybir.InstNoOp(
                            name=f"wsplit-{n}-{inst.name}",
                            engine=eng,
                            bass_nofuse=True,
                            sync_info=mybir.SyncInfo(on_wait=[w], on_update=[]),
                        ))
                    si.on_wait = [si.on_wait[-1]]
                out.append(inst)
            b.instructions = out


def _coverage() -> np.ndarray:
    reach = np.zeros(HP, np.float32)
    # count of i in [0,H) with z-4 <= i <= z
    for z in range(HP):
        lo, hi = max(z - (PS - 1), 0), min(z, H - 1)
        reach[z] = max(hi - lo + 1, 0)
    return np.outer(reach, reach)


def kernel(noisy: np.ndarray, sigma: np.ndarray) -> np.ndarray:
    noisy = np.asarray(noisy, np.float32)
    sigma = np.asarray(sigma, np.float32)
    x = (noisy / 255.0 - 0.5) / 0.5
    means = x.mean((-2, -1), keepdims=True)
    x = x - means
    P = np.pad(x, ((0, 0), (0, 0), (RAD, RAD), (RAD, RAD)), mode="reflect")
    Pb = P.astype(ml_dtypes.bfloat16)

    sig = float(sigma[0]) / 255.0 / 0.5
    denom = 2.0 * (C * PS * PS) * (sig * sig) + 1e-8
    key = round(-1.0 / denom, 9)
    if key not in _CACHE:
        _CACHE[key] = _build(key)
    nc = _CACHE[key]

    # bbs[(ch, rl), s, 32*s + rp] = 1 iff 0 <= rl - rp < 5
    rl = np.arange(ERS)
    rp = np.arange(QR)
    band = ((rl[:, None] - rp[None, :] >= 0)
            & (rl[:, None] - rp[None, :] < PS))  # [36, 32]
    bbs = np.zeros((C, ERS, 2, RH), ml_dtypes.float8_e4m3)
    for s in range(2):
        bbs[:, :, s, QR * s:QR * (s + 1)] = band[None]
    bbs = np.ascontiguousarray(bbs.reshape(PCH, 2, RH))
    # b2[r, y'] = 1 iff 0 <= y' - r < 5
    r = np.arange(RH)
    yy = np.arange(ER)
    b2 = ((yy[None, :] - r[:, None] >= 0)
          & (yy[None, :] - r[:, None] < PS)).astype(ml_dtypes.bfloat16)
    b2 = np.ascontiguousarray(b2)
    ident = np.eye(ER, dtype=ml_dtypes.bfloat16)

    # host-side shift materialization (oy-major layouts)
    ch_i = np.arange(C)[:, None, None, None]
    rl_i = np.arange(ERS)[None, :, None, None]
    oy_i = np.arange(WS)[None, None, :, None]
    s_i = np.arange(2)[None, None, None, :]
    rowsA = QR * s_i + rl_i + oy_i            # [1,36,15,2]
    yp_i = np.arange(ER)[:, None, None]
    oyB_i = np.arange(WS)[None, :, None]
    cB_i = np.arange(C)[None, None, :]

    in_maps = []
    for core in range(8):
        t, half = divmod(core, 2)
        r0 = half * RH
        p_loc = Pb[t, :, r0:r0 + PR, :]       # [C, 82, 146]
        pa_host = p_loc[ch_i, rowsA, :].reshape(PCH, WS, 2, PADHW)
        perm = [SW] + [o for o in range(WS) if o != SW]
        pa_host = np.ascontiguousarray(pa_host[:, perm])
        pb_host = np.ascontiguousarray(
            p_loc[cB_i, yp_i + oyB_i, :].transpose(0, 1, 2, 3))  # [68,15,3,146]
        in_maps.append({"pa": pa_host, "pb": pb_host, "bbs": bbs, "b2": b2,
                        "ident": ident})

    trace = bool(int(os.environ.get("KERNEL_TRACE", "0")))
    if trace:
        try:
            import antenv.axon_hooks  # noqa: F401
        except ImportError:
            # This image's antenv lacks axon_hooks; provide the hook via the
            # boot machinery so bass_utils can capture NTFF profiles.
            import types
            from trn_agent_boot.trn_boot import _ntff_profile_via_ctypes
            mod = types.ModuleType("antenv.axon_hooks")
            hook = _ntff_profile_via_ctypes("/opt/axon/libaxon_pjrt.so")
            mod.get_axon_ntff_profile_hook = lambda: hook
            sys.modules["antenv.axon_hooks"] = mod
    res = run_bass_kernel_spmd(nc, in_maps, core_ids=list(range(8)), trace=trace)
    if trace:
        print(f"HW exec time: {res.exec_time_ns} ns")
        kernel.last_exec_time_ns = res.exec_time_ns
        kernel.last_profile = res.profile_json

    full = np.zeros((T, HP, C, HP), np.float32)
    for core in range(8):
        t, half = divmod(core, 2)
        r0 = half * RH
        full[t, r0:r0 + ER] += res.results[core]["acc"]
    full = full.transpose(0, 2, 1, 3)  # [T, C, HP, HP]

    cnt = _coverage()
    deno = full / (cnt[None, None] + 1e-10)
    deno = deno[:, :, PW:PW + H, PW:PW + W]
    deno = deno + means
    return np.asarray(255.0 * (deno * 0.5 + 0.5), np.float32)


if __name__ == "__main__":
    noisy = np.load("/root/problem/noisy.npy")
    sigma = np.load("/root/problem/sigma.npy")
    out = kernel(noisy=noisy, sigma=sigma)
    expected = np.load("/root/problem/expected.npy")
    rel = np.linalg.norm(out - expected) / np.linalg.norm(expected)
    print(f"Relative error vs expected: {rel:.3e}")
